# revision 1
# baseline (speedup 1.0000x reference)
"""Grouped-expert FFN (MoE) Trainium2 kernel.

Problem: E=64 experts, each x[1024,512] @ w1[512,2048] -> +b1 -> gelu(erf)
-> @ w2[2048,512] -> +b2, rows >= valid_load[e] zeroed.

Strategy:
 - Expert parallelism over 8 cores, 8 expert slots per core.
 - Host transposes x per expert (xT [D,C]) so the device contracts over D
   with zero on-chip transposes: GEMM1 computes hT = w1.T-tiles @ xT
   (stationary w1 k/m tile, moving xT), GEMM2 computes yT = w2-tiles @ hT.
   Both biases land on the partition axis -> free via ACT activation bias.
 - Rows >= valid_load are never computed: work is chunked in 512 columns of
   C; experts are snake-dealt to (core, slot) by descending chunk count so a
   single SPMD program (per-slot chunk counts = max over cores) is
   near-perfectly load balanced. Host assembles the full output with zeros.
 - All fp32 (PE streams fp32 at 1 elem/cell/cycle, same peak as bf16).
"""

import numpy as np

import concourse.bass as bass
import concourse.bacc as bacc
import concourse.tile as tile
from concourse import mybir
from concourse.bass_utils import run_bass_kernel_spmd

E, CAP, D, H = 64, 1024, 512, 2048
N_CORES = 8
SLOTS = E // N_CORES
CHUNK = 512                      # columns of C per work unit
MAX_CHUNKS = CAP // CHUNK        # 2
KTILES1 = D // 128               # 4  (contraction tiles of GEMM1)
MTILES1 = H // 128               # 16 (output partition tiles of GEMM1)
KTILES2 = H // 128               # 16 (contraction tiles of GEMM2)
MTILES2 = D // 128               # 4  (output partition tiles of GEMM2)

F32 = mybir.dt.float32
F32R = mybir.dt.float32r      # fast PE mode: 1 cyc/row (vs 4 for fp32) at N>=256
MM_DT = F32R                  # matmul operand dtype (bitcast view)

_PROGRAM_CACHE: dict[tuple, object] = {}
LAST_RESULT = None               # test harness introspection


def _build_program(slot_widths: tuple):
    """One SPMD program; slot s runs chunks of widths slot_widths[s]."""
    nc = bacc.Bacc(None, target_bir_lowering=False)

    xt = nc.dram_tensor("xt", [SLOTS, D, CAP], F32, kind="ExternalInput")
    w1g = nc.dram_tensor("w1g", [SLOTS, D, H], F32, kind="ExternalInput")
    w2g = nc.dram_tensor("w2g", [SLOTS, H, D], F32, kind="ExternalInput")
    b1g = nc.dram_tensor("b1g", [SLOTS, 128, MTILES1], F32, kind="ExternalInput")
    b2g = nc.dram_tensor("b2g", [SLOTS, 128, MTILES2], F32, kind="ExternalInput")
    yt = nc.dram_tensor("yt", [SLOTS, D, CAP], F32, kind="ExternalOutput")

    Gelu = mybir.ActivationFunctionType.Gelu
    Ident = mybir.ActivationFunctionType.Identity

    with tile.TileContext(nc) as tc:
        with (
            tc.tile_pool(name="w1p", bufs=2) as w1p,
            tc.tile_pool(name="w2p", bufs=2) as w2p,
            tc.tile_pool(name="bp", bufs=2) as bp,
            tc.tile_pool(name="xp", bufs=3) as xp,
            tc.tile_pool(name="hp", bufs=1) as hp,
            tc.tile_pool(name="yp", bufs=2) as yp,
            tc.tile_pool(name="ps_h", bufs=4, space="PSUM") as ps_h,
            tc.tile_pool(name="ps_y", bufs=4, space="PSUM") as ps_y,
        ):
            # interleave big/small slots (widths are sorted descending by
            # slot index) so DMA demand per compute window stays even
            emit_order = [0, 7, 1, 6, 2, 5, 3, 4][:SLOTS]
            for s in emit_order:
                widths = slot_widths[s]
                if not widths:
                    continue
                w1_t = w1p.tile([128, KTILES1, H], MM_DT, tag="w1")
                nc.sync.dma_start(
                    out=w1_t, in_=w1g[s].rearrange("(k p) h -> p k h", p=128).bitcast(MM_DT)
                )
                b1_t = bp.tile([128, MTILES1], F32, tag="b1")
                nc.sync.dma_start(out=b1_t, in_=b1g[s])
                b2_t = bp.tile([128, MTILES2], F32, tag="b2")
                nc.sync.dma_start(out=b2_t, in_=b2g[s])
                # w2 rides the second HWDGE ring (ACT) and is emitted after
                # the first x chunk: it is only needed once GEMM2 starts
                w2_t = w2p.tile([128, KTILES2, D], MM_DT, tag="w2")

                xt_s = xt[s].rearrange("(k p) c -> p k c", p=128)
                yt_s = yt[s].rearrange("(m p) c -> p m c", p=128)

                for j, W in enumerate(widths):
                    cs = slice(j * CHUNK, j * CHUNK + W)
                    x_t = xp.tile([128, KTILES1, CHUNK], MM_DT, tag="x")
                    nc.sync.dma_start(
                        out=x_t[:, :, :W], in_=xt_s[:, :, cs].bitcast(MM_DT)
                    )
                    if j == 0:
                        nc.scalar.dma_start(
                            out=w2_t,
                            in_=w2g[s].rearrange("(k p) d -> p k d", p=128).bitcast(MM_DT),
                        )

                    h_t = hp.tile([128, KTILES2, CHUNK], MM_DT, tag="h")
                    for m in range(MTILES1):
                        ps = ps_h.tile([128, CHUNK], F32, tag="psh")
                        for k in range(KTILES1):
                            nc.tensor.matmul(
                                ps[:, :W],
                                lhsT=w1_t[:, k, m * 128:(m + 1) * 128],
                                rhs=x_t[:, k, :W],
                                start=(k == 0),
                                stop=(k == KTILES1 - 1),
                            )
                        nc.scalar.activation(
                            h_t[:, m, :W], ps[:, :W], Gelu, bias=b1_t[:, m:m + 1]
                        )

                    y_t = yp.tile([128, MTILES2, CHUNK], F32, tag="y")
                    for dm in range(MTILES2):
                        ps2 = ps_y.tile([128, CHUNK], F32, tag="psy")
                        for k in range(KTILES2):
                            nc.tensor.matmul(
                                ps2[:, :W],
                                lhsT=w2_t[:, k, dm * 128:(dm + 1) * 128],
                                rhs=h_t[:, k, :W],
                                start=(k == 0),
                                stop=(k == KTILES2 - 1),
                            )
                        nc.scalar.activation(
                            y_t[:, dm, :W], ps2[:, :W], Ident, bias=b2_t[:, dm:dm + 1]
                        )
                    nc.gpsimd.dma_start(out=yt_s[:, :, cs], in_=y_t[:, :, :W])

    nc.compile()
    return nc


def kernel(packed_inputs, valid_load, w1, b1, w2, b2, _trace=False, **_):
    global LAST_RESULT
    packed_inputs = np.ascontiguousarray(np.asarray(packed_inputs, np.float32))
    w1 = np.asarray(w1, np.float32)
    b1 = np.asarray(b1, np.float32)
    w2 = np.asarray(w2, np.float32)
    b2 = np.asarray(b2, np.float32)
    v = np.asarray(valid_load).astype(np.int64)

    out = np.zeros((E, CAP, D), np.float32)
    if int(v.max()) <= 0:
        return out

    # snake-deal experts (sorted by descending valid rows) into core slots;
    # sorting by v keeps per-slot maxima tight so the compile-time tail
    # width (max over the 8 cores) wastes little work
    order = np.argsort(-v, kind="stable")
    assign = np.empty((N_CORES, SLOTS), np.int64)
    for s in range(SLOTS):
        blk = order[s * N_CORES:(s + 1) * N_CORES]
        assign[:, s] = blk if s % 2 == 0 else blk[::-1]

    slot_widths = []
    for s in range(SLOTS):
        mv = int(v[assign[:, s]].max())
        if mv <= 0:
            slot_widths.append(())
            continue
        nfull = (mv - 1) // CHUNK          # full 512 chunks before the tail
        tail = mv - nfull * CHUNK
        # fp32r needs moving dim >= 256 for the 1 cycle/row fast path
        # (HW-verified: a 128 floor measured slower)
        tail = min(CHUNK, max(256, -(-tail // 32) * 32))
        slot_widths.append((CHUNK,) * nfull + (tail,))
    slot_widths = tuple(slot_widths)

    key = slot_widths
    if key not in _PROGRAM_CACHE:
        _PROGRAM_CACHE[key] = _build_program(slot_widths)
    nc = _PROGRAM_CACHE[key]

    in_maps = []
    for c in range(N_CORES):
        ids = assign[c]
        in_maps.append({
            "xt": np.ascontiguousarray(
                packed_inputs[ids].transpose(0, 2, 1)),
            "w1g": np.ascontiguousarray(w1[ids]),
            "w2g": np.ascontiguousarray(w2[ids]),
            "b1g": np.ascontiguousarray(
                b1[ids].reshape(SLOTS, MTILES1, 128).transpose(0, 2, 1)),
            "b2g": np.ascontiguousarray(
                b2[ids].reshape(SLOTS, MTILES2, 128).transpose(0, 2, 1)),
        })

    res = run_bass_kernel_spmd(nc, in_maps, list(range(N_CORES)), trace=_trace)
    LAST_RESULT = res

    for c in range(N_CORES):
        ytc = res.results[c]["yt"]
        for s in range(SLOTS):
            e = int(assign[c, s])
            ve = int(v[e])
            if ve > 0:
                out[e, :ve, :] = ytc[s, :, :ve].T
    return out



# revision 2
# speedup vs baseline: 1.2010x; 1.2010x over previous
"""Grouped-expert FFN (MoE) Trainium2 kernel.

Problem: E=64 experts, each x[1024,512] @ w1[512,2048] -> +b1 -> gelu(erf)
-> @ w2[2048,512] -> +b2, rows >= valid_load[e] zeroed.

Strategy:
 - Expert parallelism over 8 cores, 8 expert slots per core.
 - Host transposes x per expert (xT [D,C]) so the device contracts over D
   with zero on-chip transposes: GEMM1 computes hT = w1.T-tiles @ xT
   (stationary w1 k/m tile, moving xT), GEMM2 computes yT = w2-tiles @ hT.
   Both biases land on the partition axis -> free via ACT activation bias.
 - Rows >= valid_load are never computed: work is chunked in 512 columns of
   C; experts are snake-dealt to (core, slot) by descending chunk count so a
   single SPMD program (per-slot chunk counts = max over cores) is
   near-perfectly load balanced. Host assembles the full output with zeros.
 - bf16 matmul operands (PE streams bf16 at 1 elem/cell/cycle, same as
   fp32r, but weight DMA halves and fast-weight-load kicks in). PSUM
   accumulation stays fp32; biases fp32; y stored bf16 and upcast on host.
   Measured rel err ~1e-3 vs the 2e-2 gate.
"""

import numpy as np
import ml_dtypes

import concourse.bass as bass
import concourse.bacc as bacc
import concourse.tile as tile
from concourse import mybir
from concourse.bass_utils import run_bass_kernel_spmd

E, CAP, D, H = 64, 1024, 512, 2048
N_CORES = 8
SLOTS = E // N_CORES
CHUNK = 512                      # columns of C per work unit
KTILES1 = D // 128               # 4  (contraction tiles of GEMM1)
MTILES1 = H // 128               # 16 (output partition tiles of GEMM1)
KTILES2 = H // 128               # 16 (contraction tiles of GEMM2)
MTILES2 = D // 128               # 4  (output partition tiles of GEMM2)

F32 = mybir.dt.float32
BF16 = mybir.dt.bfloat16
NP_BF16 = ml_dtypes.bfloat16

_PROGRAM_CACHE: dict[tuple, object] = {}
LAST_RESULT = None               # test harness introspection


def _build_program(slot_widths: tuple):
    """One SPMD program; slot s runs chunks of widths slot_widths[s]."""
    nc = bacc.Bacc(None, target_bir_lowering=False)

    xt = nc.dram_tensor("xt", [SLOTS, D, CAP], BF16, kind="ExternalInput")
    w1g = nc.dram_tensor("w1g", [SLOTS, D, H], BF16, kind="ExternalInput")
    w2g = nc.dram_tensor("w2g", [SLOTS, H, D], BF16, kind="ExternalInput")
    b1g = nc.dram_tensor("b1g", [SLOTS, 128, MTILES1], F32, kind="ExternalInput")
    b2g = nc.dram_tensor("b2g", [SLOTS, 128, MTILES2], F32, kind="ExternalInput")
    yt = nc.dram_tensor("yt", [SLOTS, D, CAP], BF16, kind="ExternalOutput")

    Gelu = mybir.ActivationFunctionType.Gelu
    Ident = mybir.ActivationFunctionType.Identity

    with tile.TileContext(nc) as tc:
        with (
            tc.tile_pool(name="w1p", bufs=2) as w1p,
            tc.tile_pool(name="w2p", bufs=2) as w2p,
            tc.tile_pool(name="bp", bufs=2) as bp,
            tc.tile_pool(name="xp", bufs=3) as xp,
            tc.tile_pool(name="hp", bufs=2) as hp,
            tc.tile_pool(name="yp", bufs=2) as yp,
            tc.tile_pool(name="ps_h", bufs=4, space="PSUM") as ps_h,
            tc.tile_pool(name="ps_y", bufs=4, space="PSUM") as ps_y,
        ):
            # interleave big/small slots (widths are sorted descending by
            # slot index) so DMA demand per compute window stays even
            emit_order = [0, 7, 1, 6, 2, 5, 3, 4][:SLOTS]
            for s in emit_order:
                widths = slot_widths[s]
                if not widths:
                    continue
                w1_t = w1p.tile([128, KTILES1, H], BF16, tag="w1")
                nc.sync.dma_start(
                    out=w1_t, in_=w1g[s].rearrange("(k p) h -> p k h", p=128)
                )
                b1_t = bp.tile([128, MTILES1], F32, tag="b1")
                nc.sync.dma_start(out=b1_t, in_=b1g[s])
                b2_t = bp.tile([128, MTILES2], F32, tag="b2")
                nc.sync.dma_start(out=b2_t, in_=b2g[s])
                # w2 rides the second HWDGE ring (ACT) and is emitted after
                # the first x chunk: it is only needed once GEMM2 starts
                w2_t = w2p.tile([128, KTILES2, D], BF16, tag="w2")

                xt_s = xt[s].rearrange("(k p) c -> p k c", p=128)
                yt_s = yt[s].rearrange("(m p) c -> p m c", p=128)

                for j, W in enumerate(widths):
                    cs = slice(j * CHUNK, j * CHUNK + W)
                    x_t = xp.tile([128, KTILES1, CHUNK], BF16, tag="x")
                    nc.sync.dma_start(out=x_t[:, :, :W], in_=xt_s[:, :, cs])
                    if j == 0:
                        nc.scalar.dma_start(
                            out=w2_t,
                            in_=w2g[s].rearrange("(k p) d -> p k d", p=128),
                        )

                    h_t = hp.tile([128, KTILES2, CHUNK], BF16, tag="h")
                    for m in range(MTILES1):
                        ps = ps_h.tile([128, CHUNK], F32, tag="psh")
                        for k in range(KTILES1):
                            nc.tensor.matmul(
                                ps[:, :W],
                                lhsT=w1_t[:, k, m * 128:(m + 1) * 128],
                                rhs=x_t[:, k, :W],
                                start=(k == 0),
                                stop=(k == KTILES1 - 1),
                            )
                        nc.scalar.activation(
                            h_t[:, m, :W], ps[:, :W], Gelu, bias=b1_t[:, m:m + 1]
                        )

                    y_t = yp.tile([128, MTILES2, CHUNK], BF16, tag="y")
                    for dm in range(MTILES2):
                        ps2 = ps_y.tile([128, CHUNK], F32, tag="psy")
                        for k in range(KTILES2):
                            nc.tensor.matmul(
                                ps2[:, :W],
                                lhsT=w2_t[:, k, dm * 128:(dm + 1) * 128],
                                rhs=h_t[:, k, :W],
                                start=(k == 0),
                                stop=(k == KTILES2 - 1),
                            )
                        nc.scalar.activation(
                            y_t[:, dm, :W], ps2[:, :W], Ident, bias=b2_t[:, dm:dm + 1]
                        )
                    nc.gpsimd.dma_start(out=yt_s[:, :, cs], in_=y_t[:, :, :W])

    nc.compile()
    return nc


def kernel(packed_inputs, valid_load, w1, b1, w2, b2, _trace=False, **_):
    global LAST_RESULT
    packed_inputs = np.ascontiguousarray(np.asarray(packed_inputs, np.float32))
    w1 = np.asarray(w1, np.float32)
    b1 = np.asarray(b1, np.float32)
    w2 = np.asarray(w2, np.float32)
    b2 = np.asarray(b2, np.float32)
    v = np.asarray(valid_load).astype(np.int64)

    out = np.zeros((E, CAP, D), np.float32)
    if int(v.max()) <= 0:
        return out

    # snake-deal experts (sorted by descending valid rows) into core slots;
    # sorting by v keeps per-slot maxima tight so the compile-time tail
    # width (max over the 8 cores) wastes little work
    order = np.argsort(-v, kind="stable")
    assign = np.empty((N_CORES, SLOTS), np.int64)
    for s in range(SLOTS):
        blk = order[s * N_CORES:(s + 1) * N_CORES]
        assign[:, s] = blk if s % 2 == 0 else blk[::-1]

    slot_widths = []
    for s in range(SLOTS):
        mv = int(v[assign[:, s]].max())
        if mv <= 0:
            slot_widths.append(())
            continue
        nfull = (mv - 1) // CHUNK          # full 512 chunks before the tail
        tail = mv - nfull * CHUNK
        tail = -(-tail // 32) * 32         # round up to 32 (DMA/PE alignment)
        slot_widths.append((CHUNK,) * nfull + (tail,))
    slot_widths = tuple(slot_widths)

    key = slot_widths
    if key not in _PROGRAM_CACHE:
        _PROGRAM_CACHE[key] = _build_program(slot_widths)
    nc = _PROGRAM_CACHE[key]

    in_maps = []
    for c in range(N_CORES):
        ids = assign[c]
        in_maps.append({
            "xt": np.ascontiguousarray(
                packed_inputs[ids].transpose(0, 2, 1)).astype(NP_BF16),
            "w1g": w1[ids].astype(NP_BF16),
            "w2g": w2[ids].astype(NP_BF16),
            "b1g": np.ascontiguousarray(
                b1[ids].reshape(SLOTS, MTILES1, 128).transpose(0, 2, 1)),
            "b2g": np.ascontiguousarray(
                b2[ids].reshape(SLOTS, MTILES2, 128).transpose(0, 2, 1)),
        })

    res = run_bass_kernel_spmd(nc, in_maps, list(range(N_CORES)), trace=_trace)
    LAST_RESULT = res

    for c in range(N_CORES):
        ytc = res.results[c]["yt"]
        for s in range(SLOTS):
            e = int(assign[c, s])
            ve = int(v[e])
            if ve > 0:
                out[e, :ve, :] = ytc[s, :, :ve].astype(np.float32).T
    return out


# revision 3
# speedup vs baseline: 1.3317x; 1.1088x over previous
"""Grouped-expert FFN (MoE) Trainium2 kernel.

Problem: E=64 experts, each x[1024,512] @ w1[512,2048] -> +b1 -> gelu(erf)
-> @ w2[2048,512] -> +b2, rows >= valid_load[e] zeroed.

Strategy:
 - Expert parallelism over 8 cores, flat "position" schedule: each expert's
   valid rows (32-rounded) are cut into <=512-column pieces; pieces are
   sorted by width and dealt 8-at-a-time into positions. All cores run the
   same program (position widths = max of its 8 pieces), each position
   loads its own expert weights, so load balance is near-optimal
   (~4800 cols/core vs 4732 ideal). Cut points are tuned by a small
   deterministic local search to minimize the group-max sum.
 - Host transposes x per piece (xT [D,W]) so the device contracts over D
   with zero on-chip transposes: GEMM1 computes hT = w1.T-tiles @ xT
   (stationary w1 k/m tile, moving xT), GEMM2 computes yT = w2-tiles @ hT.
   Both biases land on the partition axis -> free via ACT activation bias.
 - bf16 matmul operands (PE streams bf16 at 1 elem/cell/cycle, same as
   fp32r, but weight DMA halves and fast-weight-load kicks in). PSUM
   accumulation stays fp32; biases fp32; y stored bf16, upcast on host.
 - w1/w2 DMAs are split into k-slices so the first matmul starts ~2us in;
   emission interleaves wide and narrow positions so weight prefetch for
   cheap-compute positions hides under expensive ones.
"""

import numpy as np
import ml_dtypes
import random

import concourse.bass as bass
import concourse.bacc as bacc
import concourse.tile as tile
from concourse import mybir
from concourse.bass_utils import run_bass_kernel_spmd

E, CAP, D, H = 64, 1024, 512, 2048
N_CORES = 8
CHUNK = 512                      # max columns of C per position
KTILES1 = D // 128               # 4  (contraction tiles of GEMM1)
MTILES1 = H // 128               # 16 (output partition tiles of GEMM1)
KTILES2 = H // 128               # 16 (contraction tiles of GEMM2)
MTILES2 = D // 128               # 4  (output partition tiles of GEMM2)

F32 = mybir.dt.float32
BF16 = mybir.dt.bfloat16
NP_BF16 = ml_dtypes.bfloat16

_PROGRAM_CACHE: dict[tuple, object] = {}
_SCHED_CACHE: dict[tuple, tuple] = {}
LAST_RESULT = None               # test harness introspection


def _r32(x):
    return -(-int(x) // 32) * 32


def _schedule(v):
    """Cut experts into <=512-wide pieces and group into positions.

    Returns (widths, emit_order, groups) where groups[g] is a list of up to
    8 pieces (expert, col_start, width) assigned to cores 0..len-1.
    """
    key = tuple(int(x) for x in v)
    if key in _SCHED_CACHE:
        return _SCHED_CACHE[key]
    v32 = [_r32(x) for x in v]
    big = [i for i in range(E) if v32[i] > CHUNK]
    FLOOR = 60  # below ~60 cols a position is LDW/dispatch-bound anyway

    def build(cuts):
        pieces = []
        for i in range(E):
            if v32[i] == 0:
                continue
            if v32[i] <= CHUNK:
                pieces.append((v32[i], i, 0))
            else:
                c = cuts[i]
                pieces.append((c, i, 0))
                pieces.append((v32[i] - c, i, c))
        return pieces

    def cost(cuts):
        w = sorted((p[0] for p in build(cuts)), reverse=True)
        P = -(-len(w) // 8)
        w = w + [0] * (P * 8 - len(w))
        return sum(max(max(w[g * 8:(g + 1) * 8]), FLOOR) for g in range(P))

    cuts = {i: CHUNK for i in big}
    rng = random.Random(7)
    cur, curc = dict(cuts), cost(cuts)
    best, bestc = dict(cur), curc
    steps = [-32, 32, -64, 64, -96, 96, -128, 128, -192, 192]
    for _ in range(30000 if big else 0):
        i = big[rng.randrange(len(big))]
        nv = cur[i] + steps[rng.randrange(len(steps))]
        if not (32 <= nv <= CHUNK and 32 <= v32[i] - nv <= CHUNK):
            continue
        nxt = dict(cur)
        nxt[i] = nv
        c2 = cost(nxt)
        if c2 <= curc:
            cur, curc = nxt, c2
            if c2 < bestc:
                best, bestc = dict(cur), c2

    pieces = sorted(build(best), reverse=True)
    P = -(-len(pieces) // 8)
    widths, groups = [], []
    for g in range(P):
        grp = pieces[g * 8:(g + 1) * 8]
        wmax = max(p[0] for p in grp)
        if wmax <= 0:
            continue
        widths.append(wmax)
        groups.append([(e, c0, w) for (w, e, c0) in grp])
    # emission order: alternate widest/narrowest so weight prefetch for
    # cheap positions hides under expensive ones; first stays widest
    idx = list(range(len(widths)))
    order = []
    lo, hi = 0, len(idx) - 1
    while lo <= hi:
        order.append(idx[lo]); lo += 1
        if lo <= hi:
            order.append(idx[hi]); hi -= 1
    res = (tuple(widths), tuple(order), groups)
    _SCHED_CACHE[key] = res
    return res


def _build_program(widths: tuple, order: tuple):
    """One SPMD program; position p runs one chunk of widths[p] columns."""
    nc = bacc.Bacc(None, target_bir_lowering=False)
    P = len(widths)

    xt = nc.dram_tensor("xt", [P, D, CHUNK], BF16, kind="ExternalInput")
    w1g = nc.dram_tensor("w1g", [P, D, H], BF16, kind="ExternalInput")
    w2g = nc.dram_tensor("w2g", [P, H, D], BF16, kind="ExternalInput")
    b1g = nc.dram_tensor("b1g", [P, 128, MTILES1], F32, kind="ExternalInput")
    b2g = nc.dram_tensor("b2g", [P, 128, MTILES2], F32, kind="ExternalInput")
    yt = nc.dram_tensor("yt", [P, D, CHUNK], BF16, kind="ExternalOutput")

    Gelu = mybir.ActivationFunctionType.Gelu
    Ident = mybir.ActivationFunctionType.Identity

    with tile.TileContext(nc) as tc:
        with (
            tc.tile_pool(name="w1p", bufs=3) as w1p,
            tc.tile_pool(name="w2p", bufs=3) as w2p,
            tc.tile_pool(name="bp", bufs=3) as bp,
            tc.tile_pool(name="xp", bufs=3) as xp,
            tc.tile_pool(name="hp", bufs=2) as hp,
            tc.tile_pool(name="yp", bufs=2) as yp,
            tc.tile_pool(name="ps_h", bufs=4, space="PSUM") as ps_h,
            tc.tile_pool(name="ps_y", bufs=4, space="PSUM") as ps_y,
        ):
            for pi, p in enumerate(order):
                W = widths[p]
                w1_s = w1g[p].rearrange("(k p) h -> p k h", p=128)
                w2_s = w2g[p].rearrange("(k p) d -> p k d", p=128)
                xt_s = xt[p].rearrange("(k p) c -> p k c", p=128)
                yt_s = yt[p].rearrange("(m p) c -> p m c", p=128)

                w1_t = w1p.tile([128, KTILES1, H], BF16, tag="w1")
                x_t = xp.tile([128, KTILES1, CHUNK], BF16, tag="x")
                # k-sliced loads: the first matmul needs only w1[k=0]+x[k=0]
                for k in range(KTILES1):
                    nc.sync.dma_start(out=w1_t[:, k], in_=w1_s[:, k])
                    nc.sync.dma_start(
                        out=x_t[:, k, :W], in_=xt_s[:, k, :W])
                b1_t = bp.tile([128, MTILES1], F32, tag="b1")
                nc.sync.dma_start(out=b1_t, in_=b1g[p])
                b2_t = bp.tile([128, MTILES2], F32, tag="b2")
                nc.sync.dma_start(out=b2_t, in_=b2g[p])
                # w2 rides the second HWDGE ring (ACT): needed at GEMM2
                w2_t = w2p.tile([128, KTILES2, D], BF16, tag="w2")
                for q in range(0, KTILES2, 4):
                    nc.scalar.dma_start(
                        out=w2_t[:, q:q + 4], in_=w2_s[:, q:q + 4])

                h_t = hp.tile([128, KTILES2, CHUNK], BF16, tag="h")
                for m in range(MTILES1):
                    ps = ps_h.tile([128, CHUNK], F32, tag="psh")
                    for k in range(KTILES1):
                        nc.tensor.matmul(
                            ps[:, :W],
                            lhsT=w1_t[:, k, m * 128:(m + 1) * 128],
                            rhs=x_t[:, k, :W],
                            start=(k == 0),
                            stop=(k == KTILES1 - 1),
                        )
                    nc.scalar.activation(
                        h_t[:, m, :W], ps[:, :W], Gelu, bias=b1_t[:, m:m + 1]
                    )

                y_t = yp.tile([128, MTILES2, CHUNK], BF16, tag="y")
                for dm in range(MTILES2):
                    ps2 = ps_y.tile([128, CHUNK], F32, tag="psy")
                    for k in range(KTILES2):
                        nc.tensor.matmul(
                            ps2[:, :W],
                            lhsT=w2_t[:, k, dm * 128:(dm + 1) * 128],
                            rhs=h_t[:, k, :W],
                            start=(k == 0),
                            stop=(k == KTILES2 - 1),
                        )
                    nc.scalar.activation(
                        y_t[:, dm, :W], ps2[:, :W], Ident, bias=b2_t[:, dm:dm + 1]
                    )
                if pi == len(order) - 1:
                    # final store on the fast HWDGE ring: shorter end drain
                    nc.sync.dma_start(out=yt_s[:, :, :W], in_=y_t[:, :, :W])
                else:
                    nc.gpsimd.dma_start(out=yt_s[:, :, :W], in_=y_t[:, :, :W])

    nc.compile()
    return nc


def kernel(packed_inputs, valid_load, w1, b1, w2, b2, _trace=False, **_):
    global LAST_RESULT
    packed_inputs = np.ascontiguousarray(np.asarray(packed_inputs, np.float32))
    w1 = np.asarray(w1, np.float32)
    b1 = np.asarray(b1, np.float32)
    w2 = np.asarray(w2, np.float32)
    b2 = np.asarray(b2, np.float32)
    v = np.asarray(valid_load).astype(np.int64)
    v = np.clip(v, 0, CAP)

    out = np.zeros((E, CAP, D), np.float32)
    if int(v.max()) <= 0:
        return out

    widths, order, groups = _schedule(v)
    P = len(widths)

    key = (widths, order)
    if key not in _PROGRAM_CACHE:
        _PROGRAM_CACHE[key] = _build_program(widths, order)
    nc = _PROGRAM_CACHE[key]

    w1_bf = w1.astype(NP_BF16)
    w2_bf = w2.astype(NP_BF16)
    xT = packed_inputs.transpose(0, 2, 1)  # [E, D, CAP] view

    in_maps = []
    for c in range(N_CORES):
        xt_h = np.zeros((P, D, CHUNK), NP_BF16)
        w1_h = np.zeros((P, D, H), NP_BF16)
        w2_h = np.zeros((P, H, D), NP_BF16)
        b1_h = np.zeros((P, 128, MTILES1), np.float32)
        b2_h = np.zeros((P, 128, MTILES2), np.float32)
        for g in range(P):
            if c >= len(groups[g]):
                continue
            e, c0, w = groups[g][c]
            if w <= 0:
                continue
            xt_h[g, :, :w] = xT[e, :, c0:c0 + w].astype(NP_BF16)
            w1_h[g] = w1_bf[e]
            w2_h[g] = w2_bf[e]
            b1_h[g] = b1[e].reshape(MTILES1, 128).T
            b2_h[g] = b2[e].reshape(MTILES2, 128).T
        in_maps.append({"xt": xt_h, "w1g": w1_h, "w2g": w2_h,
                        "b1g": b1_h, "b2g": b2_h})

    res = run_bass_kernel_spmd(nc, in_maps, list(range(N_CORES)), trace=_trace)
    LAST_RESULT = res

    for c in range(N_CORES):
        ytc = res.results[c]["yt"]
        for g in range(P):
            if c >= len(groups[g]):
                continue
            e, c0, w = groups[g][c]
            weff = min(w, int(v[e]) - c0)
            if weff > 0:
                out[e, c0:c0 + weff, :] = (
                    ytc[g, :, :weff].astype(np.float32).T)
    return out
